# revision 1
# baseline (speedup 1.0000x reference)
"""Trainium2 Bass kernel for nn_Ensembler (nms_detection).

Contract: kernel(**inputs) takes the FULL unsharded inputs
(voxel_logits [3,64,128,128,32] f32, query_logits [3,1,64,21] f32,
sem_prob_dense [21,128,128,32] f32) and returns the FULL output
[64,128,128,32] f32.

Strategy: shard the voxel grids over the flattened voxel dimension
N = X*Y*Z across 8 NeuronCores (each core owns a contiguous slice of
N).  The QxQ IoU statistics are computed as per-shard 0/1-mask GEMMs
(fp8 DoubleRow on the tensor engine) reduced with a tiny AllReduce;
the argmax / matching / merge / keep steps are then replicated on
every core, and the merge + keep + occupancy masking are
embarrassingly parallel over the local N slice.  The data-dependent
row gather aux_v[aux_idx] is realized as indirect DMAs that read the
aux logits from DRAM with device-computed row indices.

Numerical notes:
 - all mask decisions are computed from logit signs (exact): the
   iteration-2 anchor mask uses (sig(x0)+sig(x1))/2 > 0.5 <=>
   x0 + x1 > 0, avoiding sigmoid-LUT error in the decision path.
 - sigmoid LUT (ScalarE) max abs err ~3.6e-6 affects output values
   only.

Layouts per core (NS = 65536 voxels):
 - "n-layout": [128 part, ...] with n = p*512 + j (partition-major).
 - "q-layout": [128 part = (qb, q), T cols]: chunk ci covers
   n in [ci*2T, ci*2T+2T); rows 0:64 hold q for the first T, rows
   64:128 the second T.
 - L0 is read ONCE into a persistent q-layout SBUF tile that is
   overwritten in place by the merged anchor (pass B) and consumed by
   pass C.  Masks travel through DRAM as fp8 to switch layouts.
"""

import numpy as np
import ml_dtypes

S = 3
Q = 64
X, Y, Z = 128, 128, 32
N = X * Y * Z           # 524288
C_SEM = 21
NCORES = 8
NS = N // NCORES        # 65536 voxels per core
JP = NS // 128          # 512 contiguous voxels per partition (n-layout)
T = 1024                # q-layout chunk free size
NCH = NS // (2 * T)     # 32 q-layout chunks
QC = 4                  # q rows per n-layout read chunk

_compiled = None


def _register_custom_dve_ops():
    """Register two fused DVE ops at runtime (halves the DVE op count on
    the blend/mask hot paths).  Purely additive registration in the
    concourse dve_ops tables; rows stay within the 5-bit byte-36 field."""
    import concourse.dve_ops as dve_ops
    from concourse.dve_ops import DveOp
    from concourse.dve_spec import (Spec, Src0, Src1, C0, C1, Zero, lower,
                                    _has_src1)
    from concourse.dve_uop import DveOpSpec

    if "ANT_BLEND2_K" in dve_ops._SUB_OPCODE_FOR_NAME:
        by = {op.name: op for op in dve_ops.OPS}
        return by["ANT_BLEND2_K"], by["ANT_MASKGT_K"]

    def make(name, spec):
        row = dve_ops._CUSTOM_DVE_ROW_BASE + len(dve_ops.OPS)
        assert row < 0x20
        dve_ops._SUB_OPCODE_FOR_NAME[name] = row
        shas = {}
        for ver in ("v3", "v4"):
            try:
                uops = lower(spec, ver=ver)
                shas[ver] = DveOpSpec(name=name, opcode=row, uops=uops,
                                      rd1_en=_has_src1(spec)).sha(ver)
            except Exception:
                pass
        op = DveOp(name, spec, subdim=False, uops_sha=shas)
        dve_ops.OPS.append(op)
        dve_ops.CUSTOM_DVE_SPECS[name] = spec
        return op

    blend2 = make("ANT_BLEND2_K", Spec(
        body=Src0 * C0 + Src1 * C1,
        reference=lambda in0, in1, s0, s1, imm2: (
            in0.astype(np.float32) * s0 + in1 * s1).astype(np.float32),
    ))
    maskgt = make("ANT_MASKGT_K", Spec(
        body=Zero < (Src0 + Src1 * C0),
        reference=lambda in0, in1, s0, s1, imm2: (
            (in0.astype(np.float32) + in1 * s0) > 0).astype(np.float32),
    ))
    return blend2, maskgt


def _build_program(phases=("A", "AR1", "B", "G2", "AR2", "C"), real_cc=True,
                   loop_k=None):
    import dataclasses
    import concourse.bass as bass
    import concourse.bacc as bacc
    import concourse.mybir as mybir
    import concourse.tile as tile

    phases = set(phases)
    dt = mybir.dt
    Alu = mybir.AluOpType
    Act = mybir.ActivationFunctionType
    DR = mybir.MatmulPerfMode.DoubleRow

    BLEND2, MASKGT = _register_custom_dve_ops()

    def dram_view(ap, pattern, offset_elems):
        """Raw [step,count] (element units) view of a DRAM tensor AP."""
        return dataclasses.replace(ap, ap=[list(p) for p in pattern],
                                   offset=offset_elems)

    nc = bacc.Bacc("TRN2", target_bir_lowering=False, debug=False,
                   num_devices=NCORES)

    l0 = nc.dram_tensor("l0", [Q, NS], dt.float32, kind="ExternalInput").ap()
    l1 = nc.dram_tensor("l1", [Q, NS], dt.float32, kind="ExternalInput").ap()
    l2 = nc.dram_tensor("l2", [Q, NS], dt.bfloat16,
                        kind="ExternalInput").ap()
    sem = nc.dram_tensor("sem", [C_SEM, NS], dt.float32,
                         kind="ExternalInput").ap()
    revcnt = nc.dram_tensor("revcnt", [Q, Q], dt.float32,
                            kind="ExternalInput").ap()
    iotap = nc.dram_tensor("iotap", [128, 1], dt.float32,
                           kind="ExternalInput").ap()
    id64 = nc.dram_tensor("id64", [Q, Q], dt.float32,
                          kind="ExternalInput").ap()
    out = nc.dram_tensor("out", [Q, NS], dt.bfloat16,
                         kind="ExternalOutput").ap()

    import contextlib

    with tile.TileContext(nc) as tc:
        with (tc.For_i(0, loop_k, 1) if loop_k else
              contextlib.nullcontext()):
            _body(nc, tc, phases, real_cc, dram_view,
                  (l0, l1, l2, sem, revcnt, iotap, id64, out),
                  (BLEND2, MASKGT), mybir)
    nc.compile()
    return nc


def _body(nc, tc, phases, real_cc, dram_view, tensors, custom_ops, mybir):
    import dataclasses
    import concourse.bass as bass

    dt = mybir.dt
    Alu = mybir.AluOpType
    Act = mybir.ActivationFunctionType
    DR = mybir.MatmulPerfMode.DoubleRow
    l0, l1, l2, sem, revcnt, iotap, id64, out = tensors
    BLEND2, MASKGT = custom_ops

    if True:
        with tc.tile_pool(name="dram", bufs=1, space="DRAM") as dramp, \
             tc.tile_pool(name="psum", bufs=1, space="PSUM") as psump, \
             tc.tile_pool(name="stats", bufs=1) as stp:

            # ---- DRAM scratch ----------------------------------------
            m0_dram = dramp.tile([Q + 1, NS], dt.float8e4)
            ma2_dram = dramp.tile([Q + 1, NS], dt.float8e4)
            occ_dram = dramp.tile([1, NS], dt.float8e4)
            cc_in1 = dramp.tile([Q + 1, Q + 1], dt.float32)
            cc_out1 = dramp.tile([Q + 1, Q + 1], dt.float32)
            cc_in2 = dramp.tile([Q + 1, Q + 1], dt.float32)
            cc_out2 = dramp.tile([Q + 1, Q + 1], dt.float32)
            pack1_dram = dramp.tile([Q, 3], dt.float32)
            gate_dram = dramp.tile([1, 16], dt.float32)
            pack2_dram = dramp.tile([Q, 3], dt.float32)

            # ---- small persistent stat tiles -------------------------
            revc = stp.tile([Q, Q], dt.float32)
            nc.sync.dma_start(revc[:], revcnt[:])
            iou_a1 = stp.tile([Q, 1], dt.float32)
            iou_a2 = stp.tile([Q, 1], dt.float32)
            iotp = stp.tile([128, 1], dt.float32)
            nc.sync.dma_start(iotp[:], iotap[:])
            idt = stp.tile([Q, Q], dt.float32)
            nc.sync.dma_start(idt[:], id64[:])
            bd1 = stp.tile([128, 128], dt.float32)
            bd2 = stp.tile([128, 128], dt.bfloat16)
            idxb_dram = dramp.tile([1, 2 * Q], dt.float32)
            idxb_dram2 = dramp.tile([1, 2 * Q], dt.float32)
            cb_pp = stp.tile([128, 3], dt.float32)   # [cb, matched1, 1-cb]
            c3k_pp = stp.tile([128, 3], dt.float32)  # [c3, keep, 1-c3]

            ones128 = stp.tile([128, 1], dt.float8e4)
            nc.vector.memset(ones128[:], 1.0)

            g1_ps = psump.tile([128, Q + 1], dt.float32)
            g2_ps = psump.tile([128, Q + 1], dt.float32)

            # big persistent region: holds L0 logits, then anchor2 in
            # place.  Split into 8 tiles so unit-level deps stay fine-
            # grained.  All l0 loads ride the sync ring (pure load ring).
            with tc.tile_pool(name="bigp", bufs=1) as bigp:
                l0q_tiles = []

                def l0q_slice(ci):
                    # chunk ci covers q-layout cols [ci*T, (ci+1)*T)
                    ti, off = divmod(ci * T, NS // 16)
                    return l0q_tiles[ti][:, off:off + T]

                # early-allocated streaming pool: pass-B l1c / pass-C l2c
                # tiles and the m2-fill chunks live here so their loads
                # never wait on the frees of transient phase pools
                # (stack-allocator coupling).
                workp = tc.alloc_tile_pool(name="work", bufs=1)

                # =====================================================
                # PASS A: m0 masks -> DRAM roundtrip; m1 (SBUF) -> G1
                # =====================================================
                with tc.tile_pool(name="m0p", bufs=1) as pa:
                    # l0 loads stream uninterrupted on sync; the m0
                    # masks (DVE) chase them and write back via the gpsimd
                    # (SWDGE) queue so neither blocks the load rings.
                    for b in range(8):
                        lt = bigp.tile([128, NS // 16], dt.float32,
                                       name=f"l0q_{b}")
                        l0q_tiles.append(lt)
                        for qb in range(2):
                            nc.sync.dma_start(
                                lt[qb * Q:(qb + 1) * Q, :],
                                dram_view(l0,
                                          [[NS, Q], [2 * T, 4], [1, T]],
                                          b * 4 * 2 * T + qb * T))
                    for grp in range(16):
                        m0c = pa.tile([128, 2 * T], dt.float8e4,
                                      tag="m0c", bufs=2)
                        nc.vector.tensor_scalar(
                            m0c[:],
                            l0q_tiles[grp // 2][:, (grp % 2) * 2 * T:
                                                (grp % 2 + 1) * 2 * T],
                            0.0, None, op0=Alu.is_gt)
                        for qb in range(2):
                            nc.scalar.dma_start(
                                dram_view(m0_dram,
                                          [[NS, Q], [2 * T, 2], [1, T]],
                                          grp * 4 * T + qb * T),
                                m0c[qb * Q:(qb + 1) * Q, :])
                    # m1 masks: n-layout direct to SBUF (j-major + ones col)
                    with tc.tile_pool(name="m1p", bufs=1) as pm1:
                        m1_sb = pm1.tile([128, JP, Q + 1], dt.float8e4)
                        nc.vector.memset(m1_sb[:, :, Q], 1.0)
                        # scheduler-time gate: hold the m1 stream back
                        # so l0 (whose tail the m0 roundtrip serially
                        # chases) gets the full DMA bandwidth first.
                        for qc in range(Q // 2):
                            lc = pm1.tile([128, 2, JP], dt.float32,
                                          tag="ldchunk", bufs=3)
                            src = dram_view(l1,
                                            [[JP, 128], [NS, 2], [1, JP]],
                                            qc * 2 * NS)
                            with tc.tile_wait_until(0.07):
                                nc.scalar.dma_start(lc[:], src)
                            nc.vector.tensor_scalar(
                                m1_sb[:, :, qc * 2:(qc + 1) * 2],
                                lc[:].rearrange("p q j -> p j q"), 0.0,
                                None, op0=Alu.is_gt)
                        # G1 GEMM: m0 readback in q-pieces with full-JP
                        # lines (512B elems, no sub-512B DMA penalty);
                        # each piece accumulates its own PSUM row range
                        # (matmul outs may only start at partition 0/32/64
                        # and must not cross the 64 line from base 32).
                        for q0, q1 in ((0, 32), (32, 64)):
                            m0t = pm1.tile([128, 32, JP], dt.float8e4,
                                           tag="m0t", bufs=1)
                            nq = q1 - q0
                            nc.sync.dma_start(
                                m0t[:, :nq, :],
                                dram_view(m0_dram,
                                          [[JP, 128], [NS, nq], [1, JP]],
                                          q0 * NS))
                            for j in range(JP):
                                nc.tensor.matmul(
                                    g1_ps[q0:q1, :],
                                    lhsT=m0t[:, :nq, j],
                                    rhs=m1_sb[:, j, :],
                                    start=(j == 0), stop=(j == JP - 1))
                        # ones row (m1 column sums) needs no readback
                        for j in range(JP):
                            nc.tensor.matmul(
                                g1_ps[Q:Q + 1, :], lhsT=ones128[:],
                                rhs=m1_sb[:, j, :],
                                start=(j == 0), stop=(j == JP - 1))

                # occupancy early (fills the G1/AR1 window; lands in the
                # SBUF space pm1 frees, so it naturally starts post-G1):
                # occ[n] = (max_{c>=1} sem[c,n] > sem[0,n])
                with tc.tile_pool(name="semocc", bufs=1) as po:
                    sem0 = po.tile([128, JP], dt.float32)
                    nc.sync.dma_start(
                        sem0[:], dram_view(sem, [[JP, 128], [1, JP]], 0))
                    mx = po.tile([128, JP], dt.float32)
                    nc.sync.dma_start(
                        mx[:], dram_view(sem, [[JP, 128], [1, JP]], NS))
                    for g0 in range(2, C_SEM, 5):
                        rows = min(5, C_SEM - g0)
                        semc = po.tile([128, 5, JP], dt.float32,
                                       tag="semc", bufs=1, name=f"semg{g0}")
                        nc.sync.dma_start(
                            semc[:, :rows, :],
                            dram_view(sem, [[JP, 128], [NS, rows], [1, JP]],
                                      g0 * NS))
                        for k in range(rows):
                            nc.vector.tensor_tensor(
                                mx[:], mx[:], semc[:, k, :], op=Alu.max)
                    occ_n = po.tile([128, JP], dt.float8e4)
                    nc.vector.tensor_tensor(occ_n[:], mx[:], sem0[:],
                                            op=Alu.is_gt)
                    nc.sync.dma_start(
                        dram_view(occ_dram, [[JP, 128], [1, JP]], 0),
                        occ_n[:])

                # m2 masks: n-layout via the gpsimd SWDGE queue + Pool
                # engine, so the fill streams during AR1/pass B without
                # blocking the sync/scalar rings.  Persists through G2.
                # (pm2 allocated before pb: released after it, LIFO.)
                pm2 = tc.alloc_tile_pool(name="m2p", bufs=1)
                # pass-B compute staging, opened before the m2 fill so its
                # tiles reuse semocc's space (free ~AR1) instead of the
                # m2-fill chunks (free only after the m2 masks).
                pb = tc.alloc_tile_pool(name="blend", bufs=1)
                m2_sb = pm2.tile([128, JP, Q + 1], dt.float8e4)
                nc.vector.memset(m2_sb[:, :, Q], 1.0)
                with tc.tile_pool(name="m2fill", bufs=1) as pmf:
                    for qc in range(Q // 2):
                        lc2 = pmf.tile([128, 2, JP], dt.bfloat16,
                                       tag="ld2chunk", bufs=2)
                        src = dram_view(l2, [[JP, 128], [NS, 2], [1, JP]],
                                        qc * 2 * NS)
                        nc.sync.dma_start(lc2[:], src)
                        nc.vector.tensor_scalar(
                            m2_sb[:, :, qc * 2:(qc + 1) * 2],
                            lc2[:].rearrange("p q j -> p j q"), 0.0,
                            None, op0=Alu.is_gt)

                # ---- shared stats machinery --------------------------
                def stats_round(g_ps, cc_in, cc_out, iou_a, bd, idx_dram):
                    sfx = cc_in.name
                    gs = stp.tile([Q + 1, Q + 1], dt.float32,
                                  name=f"gs_{sfx}")
                    nc.vector.tensor_copy(gs[:], g_ps[0:Q + 1, :])
                    nc.scalar.dma_start(cc_in[:], gs[:])
                    if real_cc:
                        nc.gpsimd.collective_compute(
                            "AllReduce", Alu.add,
                            replica_groups=[list(range(NCORES))],
                            ins=[cc_in.opt()], outs=[cc_out.opt()])
                    else:
                        nc.scalar.dma_start(cc_out[:], cc_in[:])
                    gr = stp.tile([Q + 1, Q + 1], dt.float32,
                                  name=f"gr_{sfx}")
                    nc.scalar.dma_start(gr[:], cc_out[:])
                    sbb = stp.tile([Q, Q], dt.float32, name=f"sbb_{sfx}")
                    row = cc_out[Q:Q + 1, 0:Q]
                    nc.scalar.dma_start(
                        sbb[:], dataclasses.replace(
                            row, ap=[[0, Q]] + [list(p) for p in row.ap[1:]]))
                    inter = gr[0:Q, 0:Q]
                    sa = gr[0:Q, Q:Q + 1]
                    u = stp.tile([Q, Q], dt.float32, name=f"u_{sfx}")
                    nc.vector.tensor_scalar(u[:], inter, sa, None,
                                            op0=Alu.subtract)
                    nc.vector.tensor_tensor(u[:], sbb[:], u[:],
                                            op=Alu.subtract)
                    nc.vector.tensor_scalar(u[:], u[:], 1.0, None,
                                            op0=Alu.max)
                    nc.vector.reciprocal(u[:], u[:])
                    iou = stp.tile([Q, Q], dt.float32, name=f"iou_{sfx}")
                    nc.vector.tensor_tensor(iou[:], inter, u[:], op=Alu.mult)
                    nc.vector.tensor_reduce(iou_a[:], iou[:],
                                            axis=mybir.AxisListType.X,
                                            op=Alu.max)
                    matched = stp.tile([Q, 1], dt.float32, name=f"mt_{sfx}")
                    nc.vector.tensor_scalar(matched[:], iou_a[:], 0.2, None,
                                            op0=Alu.is_gt)
                    eq = stp.tile([Q, Q], dt.float32, name=f"eq_{sfx}")
                    nc.vector.tensor_scalar(eq[:], iou[:], iou_a[:, 0:1],
                                            None, op0=Alu.is_equal)
                    nc.vector.tensor_tensor(eq[:], eq[:], revc[:],
                                            op=Alu.mult)
                    sm = stp.tile([Q, 1], dt.float32, name=f"sm_{sfx}")
                    nc.vector.tensor_reduce(sm[:], eq[:],
                                            axis=mybir.AxisListType.X,
                                            op=Alu.max)
                    # block-diagonal one-hot gather matrix bd = [ohT; ohT]
                    # built on the PE (transpose into g_ps's free rows --
                    # g_ps is dead after the gs copy above), replacing the
                    # DRAM index-broadcast roundtrip.
                    oh = stp.tile([Q, Q], dt.float32, name=f"oh_{sfx}")
                    nc.vector.tensor_scalar(oh[:], eq[:], sm[:, 0:1],
                                            None, op0=Alu.is_equal)
                    # transpose outputs must start at PSUM partition 0;
                    # replicate to partitions 64:127 with a plain matmul
                    nc.tensor.transpose(g_ps[0:Q, 0:Q], oh[:], idt[:])
                    oht = stp.tile([Q, Q], dt.float32, name=f"oht_{sfx}")
                    nc.vector.tensor_copy(oht[:], g_ps[0:Q, 0:Q])
                    nc.tensor.matmul(g_ps[Q:2 * Q, 0:Q], lhsT=idt[:],
                                     rhs=oht[:], start=True, stop=True)
                    nc.vector.memset(bd[:], 0.0)
                    nc.vector.tensor_copy(bd[0:Q, 0:Q], oht[:])
                    nc.vector.tensor_copy(bd[Q:2 * Q, Q:2 * Q],
                                          g_ps[Q:2 * Q, 0:Q])
                    return matched

                if "AR1" in phases:
                    matched1 = stats_round(g1_ps, cc_in1, cc_out1, iou_a1,
                                           bd1, idxb_dram)
                    cb64 = stp.tile([Q, 3], dt.float32)
                    nc.vector.tensor_scalar(cb64[:, 0:1], matched1[:], 0.5,
                                            None, op0=Alu.mult)
                    nc.vector.tensor_copy(cb64[:, 1:2], matched1[:])
                    nc.vector.tensor_scalar(cb64[:, 2:3], matched1[:], -0.5,
                                            1.0, op0=Alu.mult, op1=Alu.add)
                    nc.tensor.matmul(g1_ps[0:Q, 0:3], lhsT=idt[:],
                                     rhs=cb64[:], start=True, stop=True)
                    nc.tensor.matmul(g1_ps[Q:2 * Q, 0:3], lhsT=idt[:],
                                     rhs=cb64[:], start=True, stop=True)
                    nc.vector.tensor_copy(cb_pp[:], g1_ps[0:128, 0:3])

                # =====================================================
                # PASS B: anchor2 blend in place + ma2 mask; G2 GEMM
                # =====================================================
                if "B" in phases:
                    for ci in range(NCH):   # 1024-wide chunks
                        sl = l0q_slice(ci)
                        l1c = workp.tile([128, T], dt.float32,
                                         tag="l1c", bufs=2)
                        with tc.tile_wait_until(0.08):
                            nc.scalar.dma_start(
                                l1c[:],
                                dram_view(l1, [[T, 2], [NS, Q], [1, T]],
                                          ci * 2 * T))
                        # gather logits on PE: lg = blockdiag(sel1) @ l1c
                        lg = psump.tile([128, T], dt.float32,
                                        tag="gps", bufs=3, name=f"lg_{ci}")
                        for k in range(2):
                            ks = slice(k * 512, (k + 1) * 512)
                            nc.tensor.matmul(lg[:, ks], lhsT=bd1[:],
                                             rhs=l1c[:, ks],
                                             start=True, stop=True)
                        # exact mask (l0 + matched1*l1g) > 0 (logits!)
                        if ci % 2 == 0:
                            ma2st = pb.tile([128, 2 * T], dt.float8e4,
                                            tag="ma2st", bufs=2)
                        nc.vector._custom_dve(
                            MASKGT,
                            out=ma2st[:, (ci % 2) * T:(ci % 2 + 1) * T],
                            in0=sl, in1=lg[:], s0=cb_pp[:, 1:2])
                        if ci % 2 == 1:
                            grp = ci // 2
                            for qb in range(2):
                                nc.sync.dma_start(
                                    dram_view(
                                        ma2_dram,
                                        [[NS, Q], [2 * T, 2], [1, T]],
                                        grp * 4 * T + qb * T),
                                    ma2st[qb * Q:(qb + 1) * Q, :])
                        p0c = pb.tile([128, T], dt.bfloat16, tag="p0c",
                                      bufs=3)
                        nc.scalar.activation(p0c[:], sl, Act.Sigmoid)
                        p1g = pb.tile([128, T], dt.bfloat16, tag="p1g",
                                      bufs=3)
                        nc.scalar.activation(p1g[:], lg[:], Act.Sigmoid)
                        # anchor2 = (1-cb)*p0 + cb*p1g, in place
                        nc.vector._custom_dve(
                            BLEND2, out=sl, in0=p0c[:], in1=p1g[:],
                            s0=cb_pp[:, 2:3], s1=cb_pp[:, 0:1])
                    pb.release()

                    if "G2" in phases:
                        with tc.tile_pool(name="g2", bufs=1) as pg:
                            for q0, q1 in ((0, 32), (32, 64)):
                                ma2t = pg.tile([128, 32, JP], dt.float8e4,
                                               tag="ma2t", bufs=2)
                                nq = q1 - q0
                                with tc.high_priority(offset=1500):
                                    nc.sync.dma_start(
                                        ma2t[:, :nq, :],
                                        dram_view(
                                            ma2_dram,
                                            [[JP, 128], [NS, nq], [1, JP]],
                                            q0 * NS))
                                    for j in range(JP):
                                        nc.tensor.matmul(
                                            g2_ps[q0:q1, :],
                                            lhsT=ma2t[:, :nq, j],
                                            rhs=m2_sb[:, j, :],
                                            start=(j == 0),
                                            stop=(j == JP - 1))
                            with tc.high_priority(offset=1500):
                                for j in range(JP):
                                    nc.tensor.matmul(
                                        g2_ps[Q:Q + 1, :], lhsT=ones128[:],
                                        rhs=m2_sb[:, j, :],
                                        start=(j == 0), stop=(j == JP - 1))
                    pm2.release()

                    if "AR2" in phases:
                        matched2 = stats_round(g2_ps, cc_in2, cc_out2,
                                               iou_a2, bd2, idxb_dram2)
                        pk = stp.tile([Q, 3], dt.float32)
                        nc.vector.tensor_scalar(pk[:, 0:1], matched2[:],
                                                1.0 / 3.0, None,
                                                op0=Alu.mult)
                        nc.vector.tensor_scalar(pk[:, 2:3], matched2[:],
                                                -1.0 / 3.0, 1.0,
                                                op0=Alu.mult, op1=Alu.add)
                        t64 = stp.tile([Q, 1], dt.float32)
                        nc.vector.tensor_tensor(t64[:], iou_a1[:],
                                                iou_a2[:], op=Alu.add)
                        nc.vector.tensor_scalar(pk[:, 1:2], t64[:], 0.5,
                                                0.2, op0=Alu.mult,
                                                op1=Alu.is_gt)
                        nc.tensor.matmul(g2_ps[0:Q, 0:3], lhsT=idt[:],
                                         rhs=pk[:], start=True, stop=True)
                        nc.tensor.matmul(g2_ps[Q:2 * Q, 0:3], lhsT=idt[:],
                                         rhs=pk[:], start=True, stop=True)
                        nc.vector.tensor_copy(c3k_pp[:],
                                              g2_ps[0:128, 0:3])

                    # =================================================
                    # PASS C: final merge + keep + occupancy -> out
                    # =================================================
                    if "C" in phases:
                        with tc.tile_pool(name="passc", bufs=1) as pc:
                            occ_all = pc.tile([128, NS // 2], dt.float8e4)
                            for qb in range(2):
                                nc.scalar.dma_start(
                                    occ_all[qb * Q:(qb + 1) * Q, :],
                                    dram_view(
                                        occ_dram,
                                        [[0, Q], [2 * T, NCH], [1, T]],
                                        qb * T))
                            for ci in range(NCH):
                                a2s = l0q_slice(ci)
                                l2c = workp.tile([128, T], dt.bfloat16,
                                                 tag="l1c", bufs=2)
                                nc.sync.dma_start(
                                    l2c[:],
                                    dram_view(l2,
                                              [[T, 2], [NS, Q], [1, T]],
                                              ci * 2 * T))
                                lg2 = psump.tile([128, T], dt.float32,
                                                 tag="gps", bufs=3,
                                                 name=f"lg2_{ci}")
                                for k in range(2):
                                    ks = slice(k * 512, (k + 1) * 512)
                                    nc.tensor.matmul(lg2[:, ks], lhsT=bd2[:],
                                                     rhs=l2c[:, ks],
                                                     start=True, stop=True)
                                p2g = pc.tile([128, T], dt.bfloat16,
                                              tag="p2g", bufs=3)
                                nc.scalar.activation(p2g[:], lg2[:],
                                                     Act.Sigmoid)
                                sm2 = pc.tile([128, T], dt.bfloat16,
                                              tag="sm2", bufs=3)
                                nc.vector._custom_dve(
                                    BLEND2, out=sm2[:], in0=a2s,
                                    in1=p2g[:], s0=c3k_pp[:, 2:3],
                                    s1=c3k_pp[:, 0:1])
                                oc = pc.tile([128, T], dt.bfloat16,
                                             tag="oc", bufs=3)
                                nc.vector.scalar_tensor_tensor(
                                    oc[:], sm2[:], c3k_pp[:, 1:2],
                                    occ_all[:, ci * T:(ci + 1) * T],
                                    op0=Alu.mult, op1=Alu.mult)
                                nc.scalar.dma_start(
                                    dram_view(out,
                                              [[T, 2], [NS, Q], [1, T]],
                                              ci * 2 * T),
                                    oc[:])

                if "B" not in phases:
                    pb.release()
                    pm2.release()
                workp.release()
            if "C" not in phases:
                dbg = stp.tile([Q, Q], dt.bfloat16, name="dbg_out")
                nc.vector.tensor_copy(dbg[:], revc[:])
                nc.sync.dma_start(
                    dram_view(out, [[NS, Q], [1, Q]], 0), dbg[:])


def _get_program():
    global _compiled
    if _compiled is None:
        _compiled = _build_program()
    return _compiled


def _make_in_maps(voxel_logits, sem_prob_dense):
    vl = np.ascontiguousarray(
        np.asarray(voxel_logits, dtype=np.float32).reshape(S, Q, N))
    sp = np.ascontiguousarray(
        np.asarray(sem_prob_dense, dtype=np.float32).reshape(C_SEM, N))
    revcnt = np.tile((Q - np.arange(Q, dtype=np.float32))[None, :], (Q, 1))
    iotap = np.arange(128, dtype=np.float32)[:, None]
    id64 = np.eye(Q, dtype=np.float32)
    in_maps = []
    for c in range(NCORES):
        sl = slice(c * NS, (c + 1) * NS)
        in_maps.append({
            "l0": np.ascontiguousarray(vl[0, :, sl]),
            "l1": np.ascontiguousarray(vl[1, :, sl]),
            "l2": np.ascontiguousarray(
                vl[2, :, sl]).astype(ml_dtypes.bfloat16),
            "sem": np.ascontiguousarray(sp[:, sl]),
            "revcnt": revcnt,
            "iotap": iotap,
            "id64": id64,
        })
    return in_maps


def profile_run(inputs):
    """Run once with NTFF tracing; returns exec_time_ns or None."""
    from concourse.bass_utils import run_bass_kernel_spmd

    nc = _get_program()
    in_maps = _make_in_maps(inputs["voxel_logits"], inputs["sem_prob_dense"])
    res = run_bass_kernel_spmd(nc, in_maps, list(range(NCORES)), trace=True)
    return res.exec_time_ns


def kernel(voxel_logits, query_logits, sem_prob_dense):
    from concourse.bass_utils import run_bass_kernel_spmd

    nc = _get_program()
    in_maps = _make_in_maps(voxel_logits, sem_prob_dense)
    res = run_bass_kernel_spmd(nc, in_maps, list(range(NCORES)))
    full = np.concatenate(
        [np.asarray(res.results[c]["out"]).astype(np.float32)
         for c in range(NCORES)], axis=1)
    return full.reshape(Q, X, Y, Z)



# revision 19
# speedup vs baseline: 2.1009x; 2.1009x over previous
"""Trainium2 Bass kernel for nn_Ensembler (nms_detection).

Contract: kernel(**inputs) takes the FULL unsharded inputs
(voxel_logits [3,64,128,128,32] f32, query_logits [3,1,64,21] f32,
sem_prob_dense [21,128,128,32] f32) and returns the FULL output
[64,128,128,32] f32.

Strategy: shard the voxel grids over the flattened voxel dimension
N = X*Y*Z across 8 NeuronCores (each core owns a contiguous slice of
N).  The QxQ IoU statistics are computed as per-shard 0/1-mask GEMMs
(fp8 DoubleRow on the tensor engine) reduced with a tiny AllReduce;
the argmax / matching / merge / keep steps are then replicated on
every core, and the merge + keep + occupancy masking are
embarrassingly parallel over the local N slice.  The data-dependent
row gather aux_v[aux_idx] is realized as indirect DMAs that read the
aux logits from DRAM with device-computed row indices.

Numerical notes:
 - all mask decisions are computed from logit signs (exact): the
   iteration-2 anchor mask uses (sig(x0)+sig(x1))/2 > 0.5 <=>
   x0 + x1 > 0, avoiding sigmoid-LUT error in the decision path.
 - sigmoid LUT (ScalarE) max abs err ~3.6e-6 affects output values
   only.

Layouts per core (NS = 65536 voxels):
 - "n-layout": [128 part, ...] with n = p*512 + j (partition-major).
 - "q-layout": [128 part = (qb, q), T cols]: chunk ci covers
   n in [ci*2T, ci*2T+2T); rows 0:64 hold q for the first T, rows
   64:128 the second T.
 - L0 is read ONCE into a persistent q-layout SBUF tile that is
   overwritten in place by the merged anchor (pass B) and consumed by
   pass C.  Masks travel through DRAM as fp8 to switch layouts.
"""

import numpy as np
import ml_dtypes

S = 3
Q = 64
X, Y, Z = 128, 128, 32
N = X * Y * Z           # 524288
C_SEM = 21
NCORES = 8
NS = N // NCORES        # 65536 voxels per core
JP = NS // 128          # 512 contiguous voxels per partition (n-layout)
T = 1024                # q-layout chunk free size
NCH = NS // (2 * T)     # 32 q-layout chunks
QC = 4                  # q rows per n-layout read chunk

_compiled = None


def _register_custom_dve_ops():
    """Register two fused DVE ops at runtime (halves the DVE op count on
    the blend/mask hot paths).  Purely additive registration in the
    concourse dve_ops tables; rows stay within the 5-bit byte-36 field."""
    import concourse.dve_ops as dve_ops
    from concourse.dve_ops import DveOp
    from concourse.dve_spec import (Spec, Src0, Src1, C0, C1, Zero, lower,
                                    _has_src1)
    from concourse.dve_uop import DveOpSpec

    if "ANT_BLEND2_K" in dve_ops._SUB_OPCODE_FOR_NAME:
        by = {op.name: op for op in dve_ops.OPS}
        return by["ANT_BLEND2_K"], by["ANT_MASKGT_K"]

    def make(name, spec):
        row = dve_ops._CUSTOM_DVE_ROW_BASE + len(dve_ops.OPS)
        assert row < 0x20
        dve_ops._SUB_OPCODE_FOR_NAME[name] = row
        shas = {}
        for ver in ("v3", "v4"):
            try:
                uops = lower(spec, ver=ver)
                shas[ver] = DveOpSpec(name=name, opcode=row, uops=uops,
                                      rd1_en=_has_src1(spec)).sha(ver)
            except Exception:
                pass
        op = DveOp(name, spec, subdim=False, uops_sha=shas)
        dve_ops.OPS.append(op)
        dve_ops.CUSTOM_DVE_SPECS[name] = spec
        return op

    blend2 = make("ANT_BLEND2_K", Spec(
        body=Src0 * C0 + Src1 * C1,
        reference=lambda in0, in1, s0, s1, imm2: (
            in0.astype(np.float32) * s0 + in1 * s1).astype(np.float32),
    ))
    maskgt = make("ANT_MASKGT_K", Spec(
        body=Zero < (Src0 + Src1 * C0),
        reference=lambda in0, in1, s0, s1, imm2: (
            (in0.astype(np.float32) + in1 * s0) > 0).astype(np.float32),
    ))
    return blend2, maskgt


def _build_program(phases=("A", "AR1", "B", "G2", "AR2", "C"), real_cc=True,
                   loop_k=None):
    import dataclasses
    import concourse.bass as bass
    import concourse.bacc as bacc
    import concourse.mybir as mybir
    import concourse.tile as tile

    phases = set(phases)
    dt = mybir.dt
    Alu = mybir.AluOpType
    Act = mybir.ActivationFunctionType
    DR = mybir.MatmulPerfMode.DoubleRow

    BLEND2, MASKGT = _register_custom_dve_ops()

    def dram_view(ap, pattern, offset_elems):
        """Raw [step,count] (element units) view of a DRAM tensor AP."""
        return dataclasses.replace(ap, ap=[list(p) for p in pattern],
                                   offset=offset_elems)

    nc = bacc.Bacc("TRN2", target_bir_lowering=False, debug=False,
                   num_devices=NCORES)

    l0 = nc.dram_tensor("l0", [Q, NS], dt.float32, kind="ExternalInput").ap()
    l1 = nc.dram_tensor("l1", [Q, NS], dt.float32, kind="ExternalInput").ap()
    l1b = nc.dram_tensor("l1b", [Q, NS], dt.bfloat16,
                         kind="ExternalInput").ap()
    l2 = nc.dram_tensor("l2", [Q, NS], dt.bfloat16,
                        kind="ExternalInput").ap()
    sem = nc.dram_tensor("sem", [C_SEM, NS], dt.float32,
                         kind="ExternalInput").ap()
    revcnt = nc.dram_tensor("revcnt", [Q, Q], dt.float32,
                            kind="ExternalInput").ap()
    iotap = nc.dram_tensor("iotap", [128, 1], dt.float32,
                           kind="ExternalInput").ap()
    id64 = nc.dram_tensor("id64", [Q, Q], dt.float32,
                          kind="ExternalInput").ap()
    out = nc.dram_tensor("out", [Q, NS], dt.bfloat16,
                         kind="ExternalOutput").ap()

    import contextlib

    with tile.TileContext(nc) as tc:
        with (tc.For_i(0, loop_k, 1) if loop_k else
              contextlib.nullcontext()):
            _body(nc, tc, phases, real_cc, dram_view,
                  (l0, l1, l1b, l2, sem, revcnt, iotap, id64, out),
                  (BLEND2, MASKGT), mybir)
    nc.compile()
    return nc


def _body(nc, tc, phases, real_cc, dram_view, tensors, custom_ops, mybir):
    import dataclasses
    import concourse.bass as bass

    dt = mybir.dt
    Alu = mybir.AluOpType
    Act = mybir.ActivationFunctionType
    DR = mybir.MatmulPerfMode.DoubleRow
    l0, l1, l1b, l2, sem, revcnt, iotap, id64, out = tensors
    BLEND2, MASKGT = custom_ops

    if True:
        with tc.tile_pool(name="dram", bufs=1, space="DRAM") as dramp, \
             tc.tile_pool(name="psum", bufs=1, space="PSUM") as psump, \
             tc.tile_pool(name="stats", bufs=1) as stp:

            # ---- DRAM scratch ----------------------------------------
            m0_dram = dramp.tile([Q + 1, NS], dt.float8e4)
            ma2_dram = dramp.tile([Q + 1, NS], dt.float8e4)
            occ_dram = dramp.tile([1, NS], dt.float8e4)
            cc_in1 = dramp.tile([Q + 1, Q + 1], dt.float32)
            cc_out1 = dramp.tile([Q + 1, Q + 1], dt.float32)
            cc_in2 = dramp.tile([Q + 1, Q + 1], dt.float32)
            cc_out2 = dramp.tile([Q + 1, Q + 1], dt.float32)
            pack1_dram = dramp.tile([Q, 3], dt.float32)
            gate_dram = dramp.tile([1, 16], dt.float32)
            pack2_dram = dramp.tile([Q, 3], dt.float32)

            # ---- small persistent stat tiles -------------------------
            revc = stp.tile([Q, Q], dt.float32)
            nc.sync.dma_start(revc[:], revcnt[:])
            iou_a1 = stp.tile([Q, 1], dt.float32)
            iou_a2 = stp.tile([Q, 1], dt.float32)
            iotp = stp.tile([128, 1], dt.float32)
            nc.sync.dma_start(iotp[:], iotap[:])
            idt = stp.tile([Q, Q], dt.float32)
            nc.sync.dma_start(idt[:], id64[:])
            bd1 = stp.tile([128, 128], dt.float32)
            bd2 = stp.tile([128, 128], dt.bfloat16)
            idxb_dram = dramp.tile([1, 2 * Q], dt.float32)
            idxb_dram2 = dramp.tile([1, 2 * Q], dt.float32)
            cb_pp = stp.tile([128, 3], dt.float32)   # [cb, matched1, 1-cb]
            c3k_pp = stp.tile([128, 3], dt.float32)  # [c3, keep, 1-c3]

            ones128 = stp.tile([128, 1], dt.float8e4)
            nc.vector.memset(ones128[:], 1.0)

            g1_ps = psump.tile([128, Q + 1], dt.float32)
            g2_ps = psump.tile([128, Q + 1], dt.float32)

            # big persistent region: holds L0 logits, then anchor2 in
            # place.  Split into 8 tiles so unit-level deps stay fine-
            # grained.  All l0 loads ride the sync ring (pure load ring).
            with tc.tile_pool(name="bigp", bufs=1) as bigp:
                l0q_tiles = []

                def l0q_slice(ci):
                    # chunk ci covers q-layout cols [ci*T, (ci+1)*T)
                    ti, off = divmod(ci * T, NS // 16)
                    return l0q_tiles[ti][:, off:off + T]

                # early-allocated streaming pool: pass-B l1c / pass-C l2c
                # tiles and the m2-fill chunks live here so their loads
                # never wait on the frees of transient phase pools
                # (stack-allocator coupling).
                workp = tc.alloc_tile_pool(name="work", bufs=1)

                # =====================================================
                # PASS A: m0 masks -> DRAM roundtrip; m1 (SBUF) -> G1
                # =====================================================
                with tc.tile_pool(name="m0p", bufs=1) as pa:
                    # l0 loads stream uninterrupted on sync; the m0
                    # masks (DVE) chase them and write back via the gpsimd
                    # (SWDGE) queue so neither blocks the load rings.
                    for b in range(8):
                        lt = bigp.tile([128, NS // 16], dt.float32,
                                       name=f"l0q_{b}")
                        l0q_tiles.append(lt)
                        for qb in range(2):
                            nc.sync.dma_start(
                                lt[qb * Q:(qb + 1) * Q, :],
                                dram_view(l0,
                                          [[NS, Q], [2 * T, 4], [1, T]],
                                          b * 4 * 2 * T + qb * T))
                    for grp in range(16):
                        m0c = pa.tile([128, 2 * T], dt.float8e4,
                                      tag="m0c", bufs=2)
                        nc.vector.tensor_scalar(
                            m0c[:],
                            l0q_tiles[grp // 2][:, (grp % 2) * 2 * T:
                                                (grp % 2 + 1) * 2 * T],
                            0.0, None, op0=Alu.is_gt)
                        for qb in range(2):
                            nc.scalar.dma_start(
                                dram_view(m0_dram,
                                          [[NS, Q], [2 * T, 2], [1, T]],
                                          grp * 4 * T + qb * T),
                                m0c[qb * Q:(qb + 1) * Q, :])
                    # m1 masks: n-layout direct to SBUF (j-major + ones col)
                    with tc.tile_pool(name="m1p", bufs=1) as pm1:
                        m1_sb = pm1.tile([128, JP, Q + 1], dt.float8e4)
                        nc.vector.memset(m1_sb[:, :, Q], 1.0)
                        # scheduler-time gate: hold the m1 stream back
                        # so l0 (whose tail the m0 roundtrip serially
                        # chases) gets the full DMA bandwidth first.
                        for qc in range(Q // 2):
                            lc = pm1.tile([128, 2, JP], dt.bfloat16,
                                          tag="ldchunk", bufs=2)
                            src = dram_view(l1b,
                                            [[JP, 128], [NS, 2], [1, JP]],
                                            qc * 2 * NS)
                            with tc.tile_wait_until(0.03):
                                nc.scalar.dma_start(lc[:], src)
                            nc.vector.tensor_scalar(
                                m1_sb[:, :, qc * 2:(qc + 1) * 2],
                                lc[:].rearrange("p q j -> p j q"), 0.0,
                                None, op0=Alu.is_gt)
                        # G1 GEMM: m0 readback in q-pieces with full-JP
                        # lines (512B elems, no sub-512B DMA penalty);
                        # each piece accumulates its own PSUM row range
                        # (matmul outs may only start at partition 0/32/64
                        # and must not cross the 64 line from base 32).
                        for q0, q1 in ((0, 32), (32, 64)):
                            m0t = pm1.tile([128, 32, JP], dt.float8e4,
                                           tag="m0t", bufs=1)
                            nq = q1 - q0
                            for qq in range(q0, q1, 16):
                                nc.sync.dma_start(
                                    m0t[:, qq - q0:qq - q0 + 16, :],
                                    dram_view(
                                        m0_dram,
                                        [[JP, 128], [NS, 16], [1, JP]],
                                        qq * NS))
                            for j in range(JP):
                                nc.tensor.matmul(
                                    g1_ps[q0:q1, :],
                                    lhsT=m0t[:, :nq, j],
                                    rhs=m1_sb[:, j, :],
                                    start=(j == 0), stop=(j == JP - 1))
                        # ones row (m1 column sums) needs no readback
                        for j in range(JP):
                            nc.tensor.matmul(
                                g1_ps[Q:Q + 1, :], lhsT=ones128[:],
                                rhs=m1_sb[:, j, :],
                                start=(j == 0), stop=(j == JP - 1))

                # occupancy early (fills the G1/AR1 window; lands in the
                # SBUF space pm1 frees, so it naturally starts post-G1):
                # occ[n] = (max_{c>=1} sem[c,n] > sem[0,n])
                with tc.tile_pool(name="semocc", bufs=1) as po:
                    sem0 = po.tile([128, JP], dt.float32)
                    nc.sync.dma_start(
                        sem0[:], dram_view(sem, [[JP, 128], [1, JP]], 0))
                    mx = po.tile([128, JP], dt.float32)
                    nc.sync.dma_start(
                        mx[:], dram_view(sem, [[JP, 128], [1, JP]], NS))
                    for g0 in range(2, C_SEM, 5):
                        rows = min(5, C_SEM - g0)
                        semc = po.tile([128, 5, JP], dt.float32,
                                       tag="semc", bufs=1, name=f"semg{g0}")
                        nc.sync.dma_start(
                            semc[:, :rows, :],
                            dram_view(sem, [[JP, 128], [NS, rows], [1, JP]],
                                      g0 * NS))
                        for k in range(rows):
                            nc.vector.tensor_tensor(
                                mx[:], mx[:], semc[:, k, :], op=Alu.max)
                    occ_n = po.tile([128, JP], dt.float8e4)
                    nc.vector.tensor_tensor(occ_n[:], mx[:], sem0[:],
                                            op=Alu.is_gt)
                    nc.sync.dma_start(
                        dram_view(occ_dram, [[JP, 128], [1, JP]], 0),
                        occ_n[:])

                # m2 masks: n-layout via the gpsimd SWDGE queue + Pool
                # engine, so the fill streams during AR1/pass B without
                # blocking the sync/scalar rings.  Persists through G2.
                # (pm2 allocated before pb: released after it, LIFO.)
                pm2 = tc.alloc_tile_pool(name="m2p", bufs=1)
                # pass-B compute staging, opened before the m2 fill so its
                # tiles reuse semocc's space (free ~AR1) instead of the
                # m2-fill chunks (free only after the m2 masks).
                pb = tc.alloc_tile_pool(name="blend", bufs=1)
                m2_sb = pm2.tile([128, JP, Q + 1], dt.float8e4)
                nc.vector.memset(m2_sb[:, :, Q], 1.0)
                with tc.tile_pool(name="m2fill", bufs=1) as pmf:
                    for qc in range(Q // 2):
                        lc2 = pmf.tile([128, 2, JP], dt.bfloat16,
                                       tag="ld2chunk", bufs=3)
                        src = dram_view(l2, [[JP, 128], [NS, 2], [1, JP]],
                                        qc * 2 * NS)
                        nc.sync.dma_start(lc2[:], src)
                        nc.vector.tensor_scalar(
                            m2_sb[:, :, qc * 2:(qc + 1) * 2],
                            lc2[:].rearrange("p q j -> p j q"), 0.0,
                            None, op0=Alu.is_gt)

                # ---- shared stats machinery --------------------------
                def stats_round(g_ps, cc_in, cc_out, iou_a, bd, idx_dram):
                    sfx = cc_in.name
                    gs = stp.tile([Q + 1, Q + 1], dt.float32,
                                  name=f"gs_{sfx}")
                    nc.vector.tensor_copy(gs[:], g_ps[0:Q + 1, :])
                    nc.scalar.dma_start(cc_in[:], gs[:])
                    if real_cc:
                        nc.gpsimd.collective_compute(
                            "AllReduce", Alu.add,
                            replica_groups=[list(range(NCORES))],
                            ins=[cc_in.opt()], outs=[cc_out.opt()])
                    else:
                        nc.scalar.dma_start(cc_out[:], cc_in[:])
                    gr = stp.tile([Q + 1, Q + 1], dt.float32,
                                  name=f"gr_{sfx}")
                    nc.scalar.dma_start(gr[:], cc_out[:])
                    sbb = stp.tile([Q, Q], dt.float32, name=f"sbb_{sfx}")
                    row = cc_out[Q:Q + 1, 0:Q]
                    nc.scalar.dma_start(
                        sbb[:], dataclasses.replace(
                            row, ap=[[0, Q]] + [list(p) for p in row.ap[1:]]))
                    inter = gr[0:Q, 0:Q]
                    sa = gr[0:Q, Q:Q + 1]
                    u = stp.tile([Q, Q], dt.float32, name=f"u_{sfx}")
                    nc.vector.tensor_scalar(u[:], inter, sa, None,
                                            op0=Alu.subtract)
                    nc.vector.tensor_tensor(u[:], sbb[:], u[:],
                                            op=Alu.subtract)
                    nc.vector.tensor_scalar(u[:], u[:], 1.0, None,
                                            op0=Alu.max)
                    nc.vector.reciprocal(u[:], u[:])
                    iou = stp.tile([Q, Q], dt.float32, name=f"iou_{sfx}")
                    nc.vector.tensor_tensor(iou[:], inter, u[:], op=Alu.mult)
                    nc.vector.tensor_reduce(iou_a[:], iou[:],
                                            axis=mybir.AxisListType.X,
                                            op=Alu.max)
                    matched = stp.tile([Q, 1], dt.float32, name=f"mt_{sfx}")
                    nc.vector.tensor_scalar(matched[:], iou_a[:], 0.2, None,
                                            op0=Alu.is_gt)
                    eq = stp.tile([Q, Q], dt.float32, name=f"eq_{sfx}")
                    nc.vector.tensor_scalar(eq[:], iou[:], iou_a[:, 0:1],
                                            None, op0=Alu.is_equal)
                    nc.vector.tensor_tensor(eq[:], eq[:], revc[:],
                                            op=Alu.mult)
                    sm = stp.tile([Q, 1], dt.float32, name=f"sm_{sfx}")
                    nc.vector.tensor_reduce(sm[:], eq[:],
                                            axis=mybir.AxisListType.X,
                                            op=Alu.max)
                    # block-diagonal one-hot gather matrix bd = [ohT; ohT]
                    # built on the PE (transpose into g_ps's free rows --
                    # g_ps is dead after the gs copy above), replacing the
                    # DRAM index-broadcast roundtrip.
                    oh = stp.tile([Q, Q], dt.float32, name=f"oh_{sfx}")
                    nc.vector.tensor_scalar(oh[:], eq[:], sm[:, 0:1],
                                            None, op0=Alu.is_equal)
                    # transpose outputs must start at PSUM partition 0;
                    # replicate to partitions 64:127 with a plain matmul
                    nc.tensor.transpose(g_ps[0:Q, 0:Q], oh[:], idt[:])
                    oht = stp.tile([Q, Q], dt.float32, name=f"oht_{sfx}")
                    nc.vector.tensor_copy(oht[:], g_ps[0:Q, 0:Q])
                    nc.tensor.matmul(g_ps[Q:2 * Q, 0:Q], lhsT=idt[:],
                                     rhs=oht[:], start=True, stop=True)
                    nc.vector.memset(bd[:], 0.0)
                    nc.vector.tensor_copy(bd[0:Q, 0:Q], oht[:])
                    nc.vector.tensor_copy(bd[Q:2 * Q, Q:2 * Q],
                                          g_ps[Q:2 * Q, 0:Q])
                    return matched

                if "AR1" in phases:
                    matched1 = stats_round(g1_ps, cc_in1, cc_out1, iou_a1,
                                           bd1, idxb_dram)
                    cb64 = stp.tile([Q, 3], dt.float32)
                    nc.vector.tensor_scalar(cb64[:, 0:1], matched1[:], 0.5,
                                            None, op0=Alu.mult)
                    nc.vector.tensor_copy(cb64[:, 1:2], matched1[:])
                    nc.vector.tensor_scalar(cb64[:, 2:3], matched1[:], -0.5,
                                            1.0, op0=Alu.mult, op1=Alu.add)
                    nc.tensor.matmul(g1_ps[0:Q, 0:3], lhsT=idt[:],
                                     rhs=cb64[:], start=True, stop=True)
                    nc.tensor.matmul(g1_ps[Q:2 * Q, 0:3], lhsT=idt[:],
                                     rhs=cb64[:], start=True, stop=True)
                    nc.vector.tensor_copy(cb_pp[:], g1_ps[0:128, 0:3])

                # =====================================================
                # PASS B: anchor2 blend in place + ma2 mask; G2 GEMM
                # =====================================================
                if "B" in phases:
                    for ci in range(NCH):   # 1024-wide chunks
                        sl = l0q_slice(ci)
                        l1c = workp.tile([128, T], dt.float32,
                                         tag="l1c", bufs=3)
                        with tc.tile_wait_until(0.05):
                            for qb in range(2):
                                nc.scalar.dma_start(
                                    l1c[qb * Q:(qb + 1) * Q, :],
                                    dram_view(l1, [[NS, Q], [1, T]],
                                              ci * 2 * T + qb * T))
                        # gather logits on PE: lg = blockdiag(sel1) @ l1c
                        lg = psump.tile([128, T], dt.float32,
                                        tag="gps", bufs=3, name=f"lg_{ci}")
                        for k in range(2):
                            ks = slice(k * 512, (k + 1) * 512)
                            nc.tensor.matmul(lg[:, ks], lhsT=bd1[:],
                                             rhs=l1c[:, ks],
                                             start=True, stop=True)
                        # exact mask (l0 + matched1*l1g) > 0 (logits!)
                        if ci % 2 == 0:
                            ma2st = pb.tile([128, 2 * T], dt.float8e4,
                                            tag="ma2st", bufs=2)
                        nc.vector._custom_dve(
                            MASKGT,
                            out=ma2st[:, (ci % 2) * T:(ci % 2 + 1) * T],
                            in0=sl, in1=lg[:], s0=cb_pp[:, 1:2])
                        if ci % 2 == 1:
                            grp = ci // 2
                            for qb in range(2):
                                nc.sync.dma_start(
                                    dram_view(
                                        ma2_dram,
                                        [[NS, Q], [2 * T, 2], [1, T]],
                                        grp * 4 * T + qb * T),
                                    ma2st[qb * Q:(qb + 1) * Q, :])
                        p0c = pb.tile([128, T], dt.bfloat16, tag="p0c",
                                      bufs=4)
                        nc.scalar.activation(p0c[:], sl, Act.Sigmoid)
                        p1g = pb.tile([128, T], dt.bfloat16, tag="p1g",
                                      bufs=4)
                        nc.scalar.activation(p1g[:], lg[:], Act.Sigmoid)
                        # anchor2 = (1-cb)*p0 + cb*p1g, in place
                        nc.vector._custom_dve(
                            BLEND2, out=sl, in0=p0c[:], in1=p1g[:],
                            s0=cb_pp[:, 2:3], s1=cb_pp[:, 0:1])
                    pb.release()

                    if "G2" in phases:
                        with tc.tile_pool(name="g2", bufs=1) as pg:
                            for q0, q1 in ((0, 32), (32, 64)):
                                ma2t = pg.tile([128, 32, JP], dt.float8e4,
                                               tag="ma2t", bufs=1)
                                nq = q1 - q0
                                with tc.high_priority(offset=1500):
                                    for qq in range(q0, q1, 16):
                                        nc.sync.dma_start(
                                            ma2t[:, qq - q0:qq - q0 + 16, :],
                                            dram_view(
                                                ma2_dram,
                                                [[JP, 128], [NS, 16],
                                                 [1, JP]],
                                                qq * NS))
                                    for j in range(JP):
                                        nc.tensor.matmul(
                                            g2_ps[q0:q1, :],
                                            lhsT=ma2t[:, :nq, j],
                                            rhs=m2_sb[:, j, :],
                                            start=(j == 0),
                                            stop=(j == JP - 1))
                            with tc.high_priority(offset=1500):
                                for j in range(JP):
                                    nc.tensor.matmul(
                                        g2_ps[Q:Q + 1, :], lhsT=ones128[:],
                                        rhs=m2_sb[:, j, :],
                                        start=(j == 0), stop=(j == JP - 1))
                    pm2.release()

                    if "AR2" in phases:
                        matched2 = stats_round(g2_ps, cc_in2, cc_out2,
                                               iou_a2, bd2, idxb_dram2)
                        pk = stp.tile([Q, 3], dt.float32)
                        nc.vector.tensor_scalar(pk[:, 0:1], matched2[:],
                                                1.0 / 3.0, None,
                                                op0=Alu.mult)
                        nc.vector.tensor_scalar(pk[:, 2:3], matched2[:],
                                                -1.0 / 3.0, 1.0,
                                                op0=Alu.mult, op1=Alu.add)
                        t64 = stp.tile([Q, 1], dt.float32)
                        nc.vector.tensor_tensor(t64[:], iou_a1[:],
                                                iou_a2[:], op=Alu.add)
                        nc.vector.tensor_scalar(pk[:, 1:2], t64[:], 0.5,
                                                0.2, op0=Alu.mult,
                                                op1=Alu.is_gt)
                        nc.tensor.matmul(g2_ps[0:Q, 0:3], lhsT=idt[:],
                                         rhs=pk[:], start=True, stop=True)
                        nc.tensor.matmul(g2_ps[Q:2 * Q, 0:3], lhsT=idt[:],
                                         rhs=pk[:], start=True, stop=True)
                        nc.vector.tensor_copy(c3k_pp[:],
                                              g2_ps[0:128, 0:3])

                    # =================================================
                    # PASS C: final merge + keep + occupancy -> out
                    # =================================================
                    if "C" in phases:
                        with tc.tile_pool(name="passc", bufs=1) as pc:
                            for ci in range(NCH):
                                a2s = l0q_slice(ci)
                                l2c = workp.tile([128, T], dt.bfloat16,
                                                 tag="l1c", bufs=3)
                                occ_c = pc.tile([128, T], dt.float8e4,
                                                tag="occ_c", bufs=4)
                                for qb in range(2):
                                    nc.sync.dma_start(
                                        l2c[qb * Q:(qb + 1) * Q, :],
                                        dram_view(l2, [[NS, Q], [1, T]],
                                                  ci * 2 * T + qb * T))
                                    nc.sync.dma_start(
                                        occ_c[qb * Q:(qb + 1) * Q, :],
                                        dram_view(occ_dram,
                                                  [[0, Q], [1, T]],
                                                  ci * 2 * T + qb * T))
                                lg2 = psump.tile([128, T], dt.float32,
                                                 tag="gps", bufs=3,
                                                 name=f"lg2_{ci}")
                                for k in range(2):
                                    ks = slice(k * 512, (k + 1) * 512)
                                    nc.tensor.matmul(lg2[:, ks], lhsT=bd2[:],
                                                     rhs=l2c[:, ks],
                                                     start=True, stop=True)
                                p2g = pc.tile([128, T], dt.bfloat16,
                                              tag="p2g", bufs=4)
                                nc.scalar.activation(p2g[:], lg2[:],
                                                     Act.Sigmoid)
                                sm2 = pc.tile([128, T], dt.bfloat16,
                                              tag="sm2", bufs=4)
                                nc.vector._custom_dve(
                                    BLEND2, out=sm2[:], in0=a2s,
                                    in1=p2g[:], s0=c3k_pp[:, 2:3],
                                    s1=c3k_pp[:, 0:1])
                                oc = pc.tile([128, T], dt.bfloat16,
                                             tag="oc", bufs=4)
                                nc.vector.scalar_tensor_tensor(
                                    oc[:], sm2[:], c3k_pp[:, 1:2],
                                    occ_c[:],
                                    op0=Alu.mult, op1=Alu.mult)
                                for qb in range(2):
                                    nc.scalar.dma_start(
                                        dram_view(out, [[NS, Q], [1, T]],
                                                  ci * 2 * T + qb * T),
                                        oc[qb * Q:(qb + 1) * Q, :])

                if "B" not in phases:
                    pb.release()
                    pm2.release()
                workp.release()
            if "C" not in phases:
                dbg = stp.tile([Q, Q], dt.bfloat16, name="dbg_out")
                nc.vector.tensor_copy(dbg[:], revc[:])
                nc.sync.dma_start(
                    dram_view(out, [[NS, Q], [1, Q]], 0), dbg[:])


def _get_program():
    global _compiled
    if _compiled is None:
        _compiled = _build_program()
    return _compiled


def _make_in_maps(voxel_logits, sem_prob_dense):
    vl = np.ascontiguousarray(
        np.asarray(voxel_logits, dtype=np.float32).reshape(S, Q, N))
    sp = np.ascontiguousarray(
        np.asarray(sem_prob_dense, dtype=np.float32).reshape(C_SEM, N))
    revcnt = np.tile((Q - np.arange(Q, dtype=np.float32))[None, :], (Q, 1))
    iotap = np.arange(128, dtype=np.float32)[:, None]
    id64 = np.eye(Q, dtype=np.float32)
    in_maps = []
    for c in range(NCORES):
        sl = slice(c * NS, (c + 1) * NS)
        in_maps.append({
            "l0": np.ascontiguousarray(vl[0, :, sl]),
            "l1": np.ascontiguousarray(vl[1, :, sl]),
            "l1b": np.ascontiguousarray(
                vl[1, :, sl]).astype(ml_dtypes.bfloat16),
            "l2": np.ascontiguousarray(
                vl[2, :, sl]).astype(ml_dtypes.bfloat16),
            "sem": np.ascontiguousarray(sp[:, sl]),
            "revcnt": revcnt,
            "iotap": iotap,
            "id64": id64,
        })
    return in_maps


def profile_run(inputs):
    """Run once with NTFF tracing; returns exec_time_ns or None."""
    from concourse.bass_utils import run_bass_kernel_spmd

    nc = _get_program()
    in_maps = _make_in_maps(inputs["voxel_logits"], inputs["sem_prob_dense"])
    res = run_bass_kernel_spmd(nc, in_maps, list(range(NCORES)), trace=True)
    return res.exec_time_ns


def kernel(voxel_logits, query_logits, sem_prob_dense):
    from concourse.bass_utils import run_bass_kernel_spmd

    nc = _get_program()
    in_maps = _make_in_maps(voxel_logits, sem_prob_dense)
    res = run_bass_kernel_spmd(nc, in_maps, list(range(NCORES)))
    full = np.concatenate(
        [np.asarray(res.results[c]["out"]).astype(np.float32)
         for c in range(NCORES)], axis=1)
    return full.reshape(Q, X, Y, Z)



# revision 30
# speedup vs baseline: 6.1941x; 2.9483x over previous
"""Trainium2 Bass kernel for nn_Ensembler (nms_detection).

Contract: kernel(**inputs) takes the FULL unsharded inputs
(voxel_logits [3,64,128,128,32] f32, query_logits [3,1,64,21] f32,
sem_prob_dense [21,128,128,32] f32) and returns the FULL output
[64,128,128,32] f32.

Strategy: shard the voxel grids over the flattened voxel dimension
N = X*Y*Z across 8 NeuronCores (each core owns a contiguous slice of
N).  The QxQ IoU statistics are computed as per-shard 0/1-mask GEMMs
(fp8 DoubleRow on the tensor engine) reduced with a tiny AllReduce;
the argmax / matching / merge / keep steps are then replicated on
every core, and the merge + keep + occupancy masking are
embarrassingly parallel over the local N slice.  The data-dependent
row gather aux_v[aux_idx] is realized as indirect DMAs that read the
aux logits from DRAM with device-computed row indices.

Numerical notes:
 - all mask decisions are computed from logit signs (exact): the
   iteration-2 anchor mask uses (sig(x0)+sig(x1))/2 > 0.5 <=>
   x0 + x1 > 0, avoiding sigmoid-LUT error in the decision path.
 - sigmoid LUT (ScalarE) max abs err ~3.6e-6 affects output values
   only.

Layouts per core (NS = 65536 voxels):
 - "n-layout": [128 part, ...] with n = p*512 + j (partition-major).
 - "q-layout": [128 part = (qb, q), T cols]: chunk ci covers
   n in [ci*2T, ci*2T+2T); rows 0:64 hold q for the first T, rows
   64:128 the second T.
 - L0 is read ONCE into a persistent q-layout SBUF tile that is
   overwritten in place by the merged anchor (pass B) and consumed by
   pass C.  Masks travel through DRAM as fp8 to switch layouts.
"""

import numpy as np
import ml_dtypes

S = 3
Q = 64
X, Y, Z = 128, 128, 32
N = X * Y * Z           # 524288
C_SEM = 21
NCORES = 8
NS = N // NCORES        # 65536 voxels per core
JP = NS // 128          # 512 contiguous voxels per partition (n-layout)
T = 1024                # q-layout chunk free size
NCH = NS // (2 * T)     # 32 q-layout chunks
QC = 4                  # q rows per n-layout read chunk

_compiled = None


def _register_custom_dve_ops():
    """Register two fused DVE ops at runtime (halves the DVE op count on
    the blend/mask hot paths).  Purely additive registration in the
    concourse dve_ops tables; rows stay within the 5-bit byte-36 field."""
    import concourse.dve_ops as dve_ops
    from concourse.dve_ops import DveOp
    from concourse.dve_spec import (Spec, Src0, Src1, C0, C1, Zero, lower,
                                    _has_src1)
    from concourse.dve_uop import DveOpSpec

    if "ANT_BLEND2_K" in dve_ops._SUB_OPCODE_FOR_NAME:
        by = {op.name: op for op in dve_ops.OPS}
        return by["ANT_BLEND2_K"], by["ANT_MASKGT_K"]

    def make(name, spec):
        row = dve_ops._CUSTOM_DVE_ROW_BASE + len(dve_ops.OPS)
        assert row < 0x20
        dve_ops._SUB_OPCODE_FOR_NAME[name] = row
        shas = {}
        for ver in ("v3", "v4"):
            try:
                uops = lower(spec, ver=ver)
                shas[ver] = DveOpSpec(name=name, opcode=row, uops=uops,
                                      rd1_en=_has_src1(spec)).sha(ver)
            except Exception:
                pass
        op = DveOp(name, spec, subdim=False, uops_sha=shas)
        dve_ops.OPS.append(op)
        dve_ops.CUSTOM_DVE_SPECS[name] = spec
        return op

    blend2 = make("ANT_BLEND2_K", Spec(
        body=Src0 * C0 + Src1 * C1,
        reference=lambda in0, in1, s0, s1, imm2: (
            in0.astype(np.float32) * s0 + in1 * s1).astype(np.float32),
    ))
    maskgt = make("ANT_MASKGT_K", Spec(
        body=Zero < (Src0 + Src1 * C0),
        reference=lambda in0, in1, s0, s1, imm2: (
            (in0.astype(np.float32) + in1 * s0) > 0).astype(np.float32),
    ))
    return blend2, maskgt


def _build_program(phases=("A", "AR1", "B", "G2", "AR2", "C"), real_cc=True,
                   loop_k=None):
    import dataclasses
    import concourse.bass as bass
    import concourse.bacc as bacc
    import concourse.mybir as mybir
    import concourse.tile as tile

    phases = set(phases)
    dt = mybir.dt
    Alu = mybir.AluOpType
    Act = mybir.ActivationFunctionType
    DR = mybir.MatmulPerfMode.DoubleRow

    BLEND2, MASKGT = _register_custom_dve_ops()

    def dram_view(ap, pattern, offset_elems):
        """Raw [step,count] (element units) view of a DRAM tensor AP."""
        return dataclasses.replace(ap, ap=[list(p) for p in pattern],
                                   offset=offset_elems)

    nc = bacc.Bacc("TRN2", target_bir_lowering=False, debug=False,
                   num_devices=NCORES)

    l0 = nc.dram_tensor("l0", [Q, NS], dt.float32, kind="ExternalInput").ap()
    l1 = nc.dram_tensor("l1", [Q, NS], dt.float32, kind="ExternalInput").ap()
    l0b = nc.dram_tensor("l0b", [Q, NS], dt.bfloat16,
                         kind="ExternalInput").ap()
    l1b = nc.dram_tensor("l1b", [Q, NS], dt.bfloat16,
                         kind="ExternalInput").ap()
    l2 = nc.dram_tensor("l2", [Q, NS], dt.bfloat16,
                        kind="ExternalInput").ap()
    sem = nc.dram_tensor("sem", [C_SEM, NS], dt.float32,
                         kind="ExternalInput").ap()
    revcnt = nc.dram_tensor("revcnt", [Q, Q], dt.float32,
                            kind="ExternalInput").ap()
    iotap = nc.dram_tensor("iotap", [128, 1], dt.float32,
                           kind="ExternalInput").ap()
    id64 = nc.dram_tensor("id64", [Q, Q], dt.float32,
                          kind="ExternalInput").ap()
    out = nc.dram_tensor("out", [Q, NS], dt.bfloat16,
                         kind="ExternalOutput").ap()

    import contextlib

    with tile.TileContext(nc) as tc:
        with (tc.For_i(0, loop_k, 1) if loop_k else
              contextlib.nullcontext()):
            _body(nc, tc, phases, real_cc, dram_view,
                  (l0, l1, l0b, l1b, l2, sem, revcnt, iotap, id64, out),
                  (BLEND2, MASKGT), mybir)
    nc.compile()
    return nc


def _body(nc, tc, phases, real_cc, dram_view, tensors, custom_ops, mybir):
    import dataclasses
    import concourse.bass as bass

    dt = mybir.dt
    Alu = mybir.AluOpType
    Act = mybir.ActivationFunctionType
    DR = mybir.MatmulPerfMode.DoubleRow
    l0, l1, l0b, l1b, l2, sem, revcnt, iotap, id64, out = tensors
    BLEND2, MASKGT = custom_ops

    if True:
        with tc.tile_pool(name="dram", bufs=1, space="DRAM") as dramp, \
             tc.tile_pool(name="psum", bufs=1, space="PSUM") as psump, \
             tc.tile_pool(name="stats", bufs=1) as stp:

            # ---- DRAM scratch ----------------------------------------
            ma2_dram = dramp.tile([Q + 1, NS], dt.float8e4)
            occ_dram = dramp.tile([1, NS], dt.float8e4)
            cc_in1 = dramp.tile([Q + 1, Q + 1], dt.float32)
            cc_out1 = dramp.tile([Q + 1, Q + 1], dt.float32)
            cc_in2 = dramp.tile([Q + 1, Q + 1], dt.float32)
            cc_out2 = dramp.tile([Q + 1, Q + 1], dt.float32)
            pack1_dram = dramp.tile([Q, 3], dt.float32)
            gate_dram = dramp.tile([1, 16], dt.float32)
            pack2_dram = dramp.tile([Q, 3], dt.float32)

            # ---- small persistent stat tiles -------------------------
            revc = stp.tile([Q, Q], dt.float32)
            nc.sync.dma_start(revc[:], revcnt[:])
            iou_a1 = stp.tile([Q, 1], dt.float32)
            iou_a2 = stp.tile([Q, 1], dt.float32)
            iotp = stp.tile([128, 1], dt.float32)
            nc.sync.dma_start(iotp[:], iotap[:])
            idt = stp.tile([Q, Q], dt.float32)
            nc.sync.dma_start(idt[:], id64[:])
            bd1 = stp.tile([128, 128], dt.float32)
            bd2 = stp.tile([128, 128], dt.bfloat16)
            idxb_dram = dramp.tile([1, 2 * Q], dt.float32)
            idxb_dram2 = dramp.tile([1, 2 * Q], dt.float32)
            cb_pp = stp.tile([128, 3], dt.float32)   # [cb, matched1, 1-cb]
            c3k_pp = stp.tile([128, 3], dt.float32)  # [c3, keep, 1-c3]

            ones128 = stp.tile([128, 1], dt.float8e4)
            nc.vector.memset(ones128[:], 1.0)

            g1_ps = psump.tile([128, Q + 1], dt.float32)
            g2_ps = psump.tile([128, Q + 1], dt.float32)

            # persistent anchor2 value region (bf16 suffices for the
            # value path; every mask/decision is computed from f32 or
            # sign-exact bf16 logit streams).
            with tc.tile_pool(name="anchp", bufs=1) as anchp:
                anch = anchp.tile([128, NS // 2], dt.bfloat16)

                # m2_sb up front (held to program end) so the l2 mask
                # fill can stream during the G1/AR1 window.
                pm2 = tc.alloc_tile_pool(name="m2p", bufs=1)
                m2_sb = pm2.tile([128, JP, Q + 1], dt.float8e4)
                nc.vector.memset(m2_sb[:, :, Q], 1.0)

                # early-allocated streaming pool: pass-B l0c/l1c, the
                # m2-fill chunks and pass-C l2c tiles live here so their
                # loads never wait on the frees of transient phase pools
                # (stack-allocator coupling).
                workp = tc.alloc_tile_pool(name="work", bufs=1)

                # =====================================================
                # PASS A: m0/m1 masks straight from bf16 n-layout reads
                # (bf16 never flips the sign of an f32, so is_gt masks
                # from the truncated copies are exact).
                # =====================================================
                with tc.tile_pool(name="maskp", bufs=1) as pa:
                    m0_sb = pa.tile([128, JP, Q], dt.float8e4)
                    m1_sb = pa.tile([128, JP, Q + 1], dt.float8e4)
                    nc.vector.memset(m1_sb[:, :, Q], 1.0)
                    for qc in range(Q // 2):
                        lc0 = pa.tile([128, 2, JP], dt.bfloat16,
                                      tag="lc0", bufs=2)
                        for qq in range(2):
                            nc.sync.dma_start(
                                lc0[:, qq, :],
                                dram_view(l0b, [[JP, 128], [1, JP]],
                                          (qc * 2 + qq) * NS))
                        nc.vector.tensor_scalar(
                            m0_sb[:, :, qc * 2:(qc + 1) * 2],
                            lc0[:].rearrange("p q j -> p j q"), 0.0,
                            None, op0=Alu.is_gt)
                        lc1 = pa.tile([128, 2, JP], dt.bfloat16,
                                      tag="lc1", bufs=2)
                        for qq in range(2):
                            nc.scalar.dma_start(
                                lc1[:, qq, :],
                                dram_view(l1b, [[JP, 128], [1, JP]],
                                          (qc * 2 + qq) * NS))
                        nc.vector.tensor_scalar(
                            m1_sb[:, :, qc * 2:(qc + 1) * 2],
                            lc1[:].rearrange("p q j -> p j q"), 0.0,
                            None, op0=Alu.is_gt)
                    # G1 GEMM straight from the SBUF masks: one 64-row
                    # LDWEIGHTS per j-slab (out partitions [0,64) are a
                    # legal PSUM range) + a 1-row ones pass for the m1
                    # column sums.
                    for j in range(JP):
                        nc.tensor.matmul(
                            g1_ps[0:Q, :],
                            lhsT=m0_sb[:, j, :],
                            rhs=m1_sb[:, j, :],
                            start=(j == 0), stop=(j == JP - 1))
                    for j in range(JP):
                        nc.tensor.matmul(
                            g1_ps[Q:Q + 1, :], lhsT=ones128[:],
                            rhs=m1_sb[:, j, :],
                            start=(j == 0), stop=(j == JP - 1))

                # occupancy early (fills the G1/AR1 window; lands in the
                # SBUF space pm1 frees, so it naturally starts post-G1):
                # occ[n] = (max_{c>=1} sem[c,n] > sem[0,n])
                with tc.tile_pool(name="semocc", bufs=1) as po:
                    sem0 = po.tile([128, JP], dt.float32)
                    nc.sync.dma_start(
                        sem0[:], dram_view(sem, [[JP, 128], [1, JP]], 0))
                    mx = po.tile([128, JP], dt.float32)
                    nc.sync.dma_start(
                        mx[:], dram_view(sem, [[JP, 128], [1, JP]], NS))
                    for g0 in range(2, C_SEM, 5):
                        rows = min(5, C_SEM - g0)
                        semc = po.tile([128, 5, JP], dt.float32,
                                       tag="semc", bufs=1, name=f"semg{g0}")
                        nc.sync.dma_start(
                            semc[:, :rows, :],
                            dram_view(sem, [[JP, 128], [NS, rows], [1, JP]],
                                      g0 * NS))
                        for k in range(rows):
                            nc.vector.tensor_tensor(
                                mx[:], mx[:], semc[:, k, :], op=Alu.max)
                    occ_n = po.tile([128, JP], dt.float8e4)
                    nc.vector.tensor_tensor(occ_n[:], mx[:], sem0[:],
                                            op=Alu.is_gt)
                    nc.sync.dma_start(
                        dram_view(occ_dram, [[JP, 128], [1, JP]], 0),
                        occ_n[:])

                # m2 mask fill: chunk ring lives in workp (no coupling
                # to pa's freed space) so the loads stream during G1/AR1.
                for qc in range(Q // 2):
                    lc2 = workp.tile([128, 2, JP], dt.bfloat16,
                                     tag="ld2chunk", bufs=3)
                    src = dram_view(l2, [[JP, 128], [NS, 2], [1, JP]],
                                    qc * 2 * NS)
                    nc.sync.dma_start(lc2[:], src)
                    nc.vector.tensor_scalar(
                        m2_sb[:, :, qc * 2:(qc + 1) * 2],
                        lc2[:].rearrange("p q j -> p j q"), 0.0,
                        None, op0=Alu.is_gt)
                # pass-B compute staging
                pb = tc.alloc_tile_pool(name="blend", bufs=1)

                # ---- shared stats machinery --------------------------
                def stats_round(g_ps, cc_in, cc_out, iou_a, bd, idx_dram):
                    sfx = cc_in.name
                    gs = stp.tile([Q + 1, Q + 1], dt.float32,
                                  name=f"gs_{sfx}")
                    nc.vector.tensor_copy(gs[:], g_ps[0:Q + 1, :])
                    nc.scalar.dma_start(cc_in[:], gs[:])
                    if real_cc:
                        nc.gpsimd.collective_compute(
                            "AllReduce", Alu.add,
                            replica_groups=[list(range(NCORES))],
                            ins=[cc_in.opt()], outs=[cc_out.opt()])
                    else:
                        nc.scalar.dma_start(cc_out[:], cc_in[:])
                    gr = stp.tile([Q + 1, Q + 1], dt.float32,
                                  name=f"gr_{sfx}")
                    nc.scalar.dma_start(gr[:], cc_out[:])
                    sbb = stp.tile([Q, Q], dt.float32, name=f"sbb_{sfx}")
                    row = cc_out[Q:Q + 1, 0:Q]
                    nc.scalar.dma_start(
                        sbb[:], dataclasses.replace(
                            row, ap=[[0, Q]] + [list(p) for p in row.ap[1:]]))
                    inter = gr[0:Q, 0:Q]
                    sa = gr[0:Q, Q:Q + 1]
                    u = stp.tile([Q, Q], dt.float32, name=f"u_{sfx}")
                    nc.vector.tensor_scalar(u[:], inter, sa, None,
                                            op0=Alu.subtract)
                    nc.vector.tensor_tensor(u[:], sbb[:], u[:],
                                            op=Alu.subtract)
                    nc.vector.tensor_scalar(u[:], u[:], 1.0, None,
                                            op0=Alu.max)
                    nc.vector.reciprocal(u[:], u[:])
                    iou = stp.tile([Q, Q], dt.float32, name=f"iou_{sfx}")
                    nc.vector.tensor_tensor(iou[:], inter, u[:], op=Alu.mult)
                    nc.vector.tensor_reduce(iou_a[:], iou[:],
                                            axis=mybir.AxisListType.X,
                                            op=Alu.max)
                    matched = stp.tile([Q, 1], dt.float32, name=f"mt_{sfx}")
                    nc.vector.tensor_scalar(matched[:], iou_a[:], 0.2, None,
                                            op0=Alu.is_gt)
                    eq = stp.tile([Q, Q], dt.float32, name=f"eq_{sfx}")
                    nc.vector.tensor_scalar(eq[:], iou[:], iou_a[:, 0:1],
                                            None, op0=Alu.is_equal)
                    nc.vector.tensor_tensor(eq[:], eq[:], revc[:],
                                            op=Alu.mult)
                    sm = stp.tile([Q, 1], dt.float32, name=f"sm_{sfx}")
                    nc.vector.tensor_reduce(sm[:], eq[:],
                                            axis=mybir.AxisListType.X,
                                            op=Alu.max)
                    # block-diagonal one-hot gather matrix bd = [ohT; ohT]
                    # built on the PE (transpose into g_ps's free rows --
                    # g_ps is dead after the gs copy above), replacing the
                    # DRAM index-broadcast roundtrip.
                    oh = stp.tile([Q, Q], dt.float32, name=f"oh_{sfx}")
                    nc.vector.tensor_scalar(oh[:], eq[:], sm[:, 0:1],
                                            None, op0=Alu.is_equal)
                    # transpose outputs must start at PSUM partition 0;
                    # replicate to partitions 64:127 with a plain matmul
                    nc.tensor.transpose(g_ps[0:Q, 0:Q], oh[:], idt[:])
                    oht = stp.tile([Q, Q], dt.float32, name=f"oht_{sfx}")
                    nc.vector.tensor_copy(oht[:], g_ps[0:Q, 0:Q])
                    nc.tensor.matmul(g_ps[Q:2 * Q, 0:Q], lhsT=idt[:],
                                     rhs=oht[:], start=True, stop=True)
                    nc.vector.memset(bd[:], 0.0)
                    nc.vector.tensor_copy(bd[0:Q, 0:Q], oht[:])
                    nc.vector.tensor_copy(bd[Q:2 * Q, Q:2 * Q],
                                          g_ps[Q:2 * Q, 0:Q])
                    return matched

                if "AR1" in phases:
                    matched1 = stats_round(g1_ps, cc_in1, cc_out1, iou_a1,
                                           bd1, idxb_dram)
                    cb64 = stp.tile([Q, 3], dt.float32)
                    nc.vector.tensor_scalar(cb64[:, 0:1], matched1[:], 0.5,
                                            None, op0=Alu.mult)
                    nc.vector.tensor_copy(cb64[:, 1:2], matched1[:])
                    nc.vector.tensor_scalar(cb64[:, 2:3], matched1[:], -0.5,
                                            1.0, op0=Alu.mult, op1=Alu.add)
                    nc.tensor.matmul(g1_ps[0:Q, 0:3], lhsT=idt[:],
                                     rhs=cb64[:], start=True, stop=True)
                    nc.tensor.matmul(g1_ps[Q:2 * Q, 0:3], lhsT=idt[:],
                                     rhs=cb64[:], start=True, stop=True)
                    nc.vector.tensor_copy(cb_pp[:], g1_ps[0:128, 0:3])

                # =====================================================
                # PASS B: anchor2 blend in place + ma2 mask; G2 GEMM
                # =====================================================
                if "B" in phases:
                    for ci in range(NCH):   # 1024-wide chunks
                        l0c = workp.tile([128, T], dt.float32,
                                         tag="l0c", bufs=3)
                        for qb in range(2):
                            nc.sync.dma_start(
                                l0c[qb * Q:(qb + 1) * Q, :],
                                dram_view(l0, [[NS, Q], [1, T]],
                                          ci * 2 * T + qb * T))
                        l1c = workp.tile([128, T], dt.float32,
                                         tag="l1c", bufs=3)
                        for qb in range(2):
                            nc.scalar.dma_start(
                                l1c[qb * Q:(qb + 1) * Q, :],
                                dram_view(l1, [[NS, Q], [1, T]],
                                          ci * 2 * T + qb * T))
                        # gather logits on PE: lg = blockdiag(sel1) @ l1c
                        lg = psump.tile([128, T], dt.float32,
                                        tag="gps", bufs=3, name=f"lg_{ci}")
                        for k in range(2):
                            ks = slice(k * 512, (k + 1) * 512)
                            nc.tensor.matmul(lg[:, ks], lhsT=bd1[:],
                                             rhs=l1c[:, ks],
                                             start=True, stop=True)
                        # exact mask (l0 + matched1*l1g) > 0 (logits!)
                        if ci % 2 == 0:
                            ma2st = pb.tile([128, 2 * T], dt.float8e4,
                                            tag="ma2st", bufs=2)
                        nc.vector._custom_dve(
                            MASKGT,
                            out=ma2st[:, (ci % 2) * T:(ci % 2 + 1) * T],
                            in0=l0c[:], in1=lg[:], s0=cb_pp[:, 1:2])
                        if ci % 2 == 1:
                            grp = ci // 2
                            for qb in range(2):
                                nc.sync.dma_start(
                                    dram_view(
                                        ma2_dram,
                                        [[NS, Q], [2 * T, 2], [1, T]],
                                        grp * 4 * T + qb * T),
                                    ma2st[qb * Q:(qb + 1) * Q, :])
                        p0c = pb.tile([128, T], dt.bfloat16, tag="p0c",
                                      bufs=4)
                        nc.scalar.activation(p0c[:], l0c[:], Act.Sigmoid)
                        p1g = pb.tile([128, T], dt.bfloat16, tag="p1g",
                                      bufs=4)
                        nc.scalar.activation(p1g[:], lg[:], Act.Sigmoid)
                        # anchor2 = (1-cb)*p0 + cb*p1g -> bf16 region
                        nc.vector._custom_dve(
                            BLEND2, out=anch[:, ci * T:(ci + 1) * T],
                            in0=p0c[:], in1=p1g[:],
                            s0=cb_pp[:, 2:3], s1=cb_pp[:, 0:1])
                    pb.release()

                    if "G2" in phases:
                        with tc.tile_pool(name="g2", bufs=1) as pg:
                            ma2t = pg.tile([128, Q, JP], dt.float8e4,
                                           tag="ma2t", bufs=1)
                            with tc.high_priority(offset=1500):
                                for q0 in range(0, Q, 16):
                                    nc.sync.dma_start(
                                        ma2t[:, q0:q0 + 16, :],
                                        dram_view(
                                            ma2_dram,
                                            [[JP, 128], [NS, 16], [1, JP]],
                                            q0 * NS))
                                for j in range(JP):
                                    nc.tensor.matmul(
                                        g2_ps[0:Q, :],
                                        lhsT=ma2t[:, :, j],
                                        rhs=m2_sb[:, j, :],
                                        start=(j == 0),
                                        stop=(j == JP - 1))
                                for j in range(JP):
                                    nc.tensor.matmul(
                                        g2_ps[Q:Q + 1, :], lhsT=ones128[:],
                                        rhs=m2_sb[:, j, :],
                                        start=(j == 0), stop=(j == JP - 1))

                    if "AR2" in phases:
                        matched2 = stats_round(g2_ps, cc_in2, cc_out2,
                                               iou_a2, bd2, idxb_dram2)
                        pk = stp.tile([Q, 3], dt.float32)
                        nc.vector.tensor_scalar(pk[:, 0:1], matched2[:],
                                                1.0 / 3.0, None,
                                                op0=Alu.mult)
                        nc.vector.tensor_scalar(pk[:, 2:3], matched2[:],
                                                -1.0 / 3.0, 1.0,
                                                op0=Alu.mult, op1=Alu.add)
                        t64 = stp.tile([Q, 1], dt.float32)
                        nc.vector.tensor_tensor(t64[:], iou_a1[:],
                                                iou_a2[:], op=Alu.add)
                        nc.vector.tensor_scalar(pk[:, 1:2], t64[:], 0.5,
                                                0.2, op0=Alu.mult,
                                                op1=Alu.is_gt)
                        nc.tensor.matmul(g2_ps[0:Q, 0:3], lhsT=idt[:],
                                         rhs=pk[:], start=True, stop=True)
                        nc.tensor.matmul(g2_ps[Q:2 * Q, 0:3], lhsT=idt[:],
                                         rhs=pk[:], start=True, stop=True)
                        nc.vector.tensor_copy(c3k_pp[:],
                                              g2_ps[0:128, 0:3])

                    # =================================================
                    # PASS C: final merge + keep + occupancy -> out
                    # =================================================
                    if "C" in phases:
                        with tc.tile_pool(name="passc", bufs=1) as pc:
                            for ci in range(NCH):
                                a2s = anch[:, ci * T:(ci + 1) * T]
                                l2c = workp.tile([128, T], dt.bfloat16,
                                                 tag="l1c", bufs=3)
                                occ_c = pc.tile([128, T], dt.float8e4,
                                                tag="occ_c", bufs=4)
                                for qb in range(2):
                                    nc.sync.dma_start(
                                        l2c[qb * Q:(qb + 1) * Q, :],
                                        dram_view(l2, [[NS, Q], [1, T]],
                                                  ci * 2 * T + qb * T))
                                    nc.sync.dma_start(
                                        occ_c[qb * Q:(qb + 1) * Q, :],
                                        dram_view(occ_dram,
                                                  [[0, Q], [1, T]],
                                                  ci * 2 * T + qb * T))
                                lg2 = psump.tile([128, T], dt.float32,
                                                 tag="gps", bufs=3,
                                                 name=f"lg2_{ci}")
                                for k in range(2):
                                    ks = slice(k * 512, (k + 1) * 512)
                                    nc.tensor.matmul(lg2[:, ks], lhsT=bd2[:],
                                                     rhs=l2c[:, ks],
                                                     start=True, stop=True)
                                p2g = pc.tile([128, T], dt.bfloat16,
                                              tag="p2g", bufs=4)
                                nc.scalar.activation(p2g[:], lg2[:],
                                                     Act.Sigmoid)
                                sm2 = pc.tile([128, T], dt.bfloat16,
                                              tag="sm2", bufs=4)
                                nc.vector._custom_dve(
                                    BLEND2, out=sm2[:], in0=a2s,
                                    in1=p2g[:], s0=c3k_pp[:, 2:3],
                                    s1=c3k_pp[:, 0:1])
                                oc = pc.tile([128, T], dt.bfloat16,
                                             tag="oc", bufs=4)
                                nc.vector.scalar_tensor_tensor(
                                    oc[:], sm2[:], c3k_pp[:, 1:2],
                                    occ_c[:],
                                    op0=Alu.mult, op1=Alu.mult)
                                for qb in range(2):
                                    nc.scalar.dma_start(
                                        dram_view(out, [[NS, Q], [1, T]],
                                                  ci * 2 * T + qb * T),
                                        oc[qb * Q:(qb + 1) * Q, :])

                if "B" not in phases:
                    pb.release()
                workp.release()
                pm2.release()
            if "C" not in phases:
                dbg = stp.tile([Q, Q], dt.bfloat16, name="dbg_out")
                nc.vector.tensor_copy(dbg[:], revc[:])
                nc.sync.dma_start(
                    dram_view(out, [[NS, Q], [1, Q]], 0), dbg[:])


def _get_program():
    global _compiled
    if _compiled is None:
        _compiled = _build_program()
    return _compiled


def _make_in_maps(voxel_logits, sem_prob_dense):
    vl = np.ascontiguousarray(
        np.asarray(voxel_logits, dtype=np.float32).reshape(S, Q, N))
    sp = np.ascontiguousarray(
        np.asarray(sem_prob_dense, dtype=np.float32).reshape(C_SEM, N))
    revcnt = np.tile((Q - np.arange(Q, dtype=np.float32))[None, :], (Q, 1))
    iotap = np.arange(128, dtype=np.float32)[:, None]
    id64 = np.eye(Q, dtype=np.float32)
    in_maps = []
    for c in range(NCORES):
        sl = slice(c * NS, (c + 1) * NS)
        in_maps.append({
            "l0": np.ascontiguousarray(vl[0, :, sl]),
            "l1": np.ascontiguousarray(vl[1, :, sl]),
            "l0b": np.ascontiguousarray(
                vl[0, :, sl]).astype(ml_dtypes.bfloat16),
            "l1b": np.ascontiguousarray(
                vl[1, :, sl]).astype(ml_dtypes.bfloat16),
            "l2": np.ascontiguousarray(
                vl[2, :, sl]).astype(ml_dtypes.bfloat16),
            "sem": np.ascontiguousarray(sp[:, sl]),
            "revcnt": revcnt,
            "iotap": iotap,
            "id64": id64,
        })
    return in_maps


def profile_run(inputs):
    """Run once with NTFF tracing; returns exec_time_ns or None."""
    from concourse.bass_utils import run_bass_kernel_spmd

    nc = _get_program()
    in_maps = _make_in_maps(inputs["voxel_logits"], inputs["sem_prob_dense"])
    res = run_bass_kernel_spmd(nc, in_maps, list(range(NCORES)), trace=True)
    return res.exec_time_ns


def kernel(voxel_logits, query_logits, sem_prob_dense):
    from concourse.bass_utils import run_bass_kernel_spmd

    nc = _get_program()
    in_maps = _make_in_maps(voxel_logits, sem_prob_dense)
    res = run_bass_kernel_spmd(nc, in_maps, list(range(NCORES)))
    full = np.concatenate(
        [np.asarray(res.results[c]["out"]).astype(np.float32)
         for c in range(NCORES)], axis=1)
    return full.reshape(Q, X, Y, Z)

